# revision 3
# baseline (speedup 1.0000x reference)
"""Trainium2 Bass kernel for nn_Classifier_64587718197982 (spiking CNN).

Network (per reference):
  3x [conv3x3(C=128, pad=1, no bias) -> TDBN (batchnorm over T,B,H,W) -> LIF]
  -> mean over (H,W) -> mean over T -> FC(128->10)

Sharding: data-parallel over batch B=32 across 8 NeuronCores (4 images/core).
TDBN mean/var become a tiny [128,2] AllReduce per layer.

Per-core kernel structure:
  conv: per image (t,b), 2 PSUM half-tiles [128,512]; 3x3 conv = 9
        range-restricted shifted matmuls (borders get fewer terms = zero pad).
        FP16 operands run at full PE rate with an 11-bit mantissa; spike
        inputs (0/1) are exact in fp16, so the only quantization is the
        weights/L1-input rounding (~2.4e-4 rel). Optional hi/lo term
        decomposition (TERMS_*) recovers ~fp32 when more margin is needed.
        PSUM -> SBUF drain on ScalarE, y -> DRAM f32; bn_stats on VectorE
        reads PSUM directly.
  stats: bn_aggr -> [mean, var]; AllReduce of [mean, var+mean^2]; then
        scale s = bn_w/sqrt(var+eps), shift d = bn_b - mean*s.
  LIF (sequential over t): xn = y*s + d (ScalarE, per-partition scale/bias);
        gate = 0.25*(1-S_prev) (GpSimd); mem = mem*gate (VectorE, in-place);
        mem += xn (GpSimd, in-place); S = (mem > 0.5) (VectorE -> fp16).
        L1 spikes stay resident in SBUF; L2 spikes spill to DRAM (SBUF space);
        L3 spikes are only pooled.
  head: feat[c,b] = sum of spikes; fc via broadcast mult (VectorE) + GpSimd
        partition all-reduce; logits = feat@fcW.T/8192 + b.
"""
import numpy as np
import ml_dtypes
from contextlib import ExitStack

import concourse.bass as bass
import concourse.mybir as mybir
import concourse.tile as tile
from concourse import bass_isa
from concourse import bacc
from concourse.bass_utils import run_bass_kernel_spmd

F32 = mybir.dt.float32
FP16 = mybir.dt.float16
AF = mybir.ActivationFunctionType
ALU = mybir.AluOpType

T, B, C, H, W = 8, 32, 128, 32, 32
NCORES = 8
BL = B // NCORES          # images per core per timestep
HW = H * W                # 1024
NIMG = T * BL             # 32 images per core
NHALF = 2                 # psum halves per image (16 rows x 32 cols = 512)
RH = H // NHALF           # rows per half
DECAY = 0.25
THRESH = 0.5
BN_EPS = 1e-5
POOL_N = float(T * HW)    # pooling divisor 8192
DEBUG = False
SIM1 = False  # single-core TimelineSim variant (skips collectives)

# fp16 hi/lo term counts per layer (1 = single fp16 term)
TERMS_L1 = 2
TERMS_L2 = 1
TERMS_L3 = 1

# conv shift order: (1,1) first so the start=True matmul covers the full tile
SHIFTS = [(1, 1)] + [(dy, dx) for dy in range(3) for dx in range(3)
                     if not (dy == 1 and dx == 1)]


def _conv_image(nc, psum_pool, ysb_pool, stats_buf, ydram, t, b,
                terms, stats_slot):
    """Emit conv for one image: terms = list of (w_sb[C,9,C], x3d[C,H,W])."""
    for h in range(NHALF):
        r_base = h * RH
        pt = psum_pool.tile([C, RH * W], F32)
        p3 = pt.rearrange("c (r w) -> c r w", r=RH)
        n_mm = len(SHIFTS) * len(terms)
        i_mm = 0
        for (dy, dx) in SHIFTS:
            oy, ox = dy - 1, dx - 1
            r0 = max(r_base, -oy)
            r1 = min(r_base + RH, H - oy)
            c0 = max(0, -ox)
            c1 = min(W, W - ox)
            k = dy * 3 + dx
            for (w_sb, x3d) in terms:
                nc.tensor.matmul(
                    p3[:, r0 - r_base:r1 - r_base, c0:c1],
                    w_sb[:, k, :],
                    x3d[:, r0 + oy:r1 + oy, c0 + ox:c1 + ox],
                    start=(i_mm == 0), stop=(i_mm == n_mm - 1),
                )
                i_mm += 1
        # stats from PSUM on VectorE; drain PSUM->SBUF on ScalarE; y -> DRAM
        nc.vector.bn_stats(out=stats_buf[:, stats_slot + h, :], in_=pt)
        ysb = ysb_pool.tile([C, RH * W], F32)
        nc.scalar.copy(out=ysb, in_=pt)
        nc.sync.dma_start(out=ydram[:, t, b, h * RH * W:(h + 1) * RH * W],
                          in_=ysb)


def _layer_stats(nc, sb1, stats_buf, bnw, bnb, eps_t, cc_in, cc_out, lname,
                 dbg_mv=None, dbg_ccr=None):
    """bn_aggr + AllReduce + scale/shift computation. Returns (s, d) tiles."""
    mv = sb1.tile([C, 2], F32, tag=f"mv{lname}")
    nc.vector.bn_aggr(out=mv, in_=stats_buf)
    if dbg_mv is not None:
        nc.sync.dma_start(out=dbg_mv[:, :], in_=mv)
    # cc payload: [mean_i, var_i + mean_i^2]
    cc = sb1.tile([C, 2], F32, tag=f"cc{lname}")
    nc.gpsimd.tensor_copy(out=cc[:, 0:1], in_=mv[:, 0:1])
    sq = sb1.tile([C, 1], F32, tag=f"sq{lname}")
    nc.vector.tensor_tensor(sq, mv[:, 0:1], mv[:, 0:1], op=ALU.mult)
    nc.vector.tensor_tensor(cc[:, 1:2], mv[:, 1:2], sq, op=ALU.add)
    nc.sync.dma_start(out=cc_in[:, :], in_=cc)
    if not SIM1:
        nc.gpsimd.collective_compute(
            "AllReduce", ALU.add,
            replica_groups=[list(range(NCORES))],
            ins=[cc_in[:, :]], outs=[cc_out[:, :]],
        )
    ccr = sb1.tile([C, 2], F32, tag=f"ccr{lname}")
    nc.sync.dma_start(out=ccr, in_=cc_in[:, :] if SIM1 else cc_out[:, :])
    if dbg_ccr is not None:
        nc.sync.dma_start(out=dbg_ccr[:, :], in_=ccr)
    nshards = 1 if SIM1 else NCORES
    mean = sb1.tile([C, 1], F32, tag=f"mean{lname}")
    nc.vector.tensor_scalar(mean, ccr[:, 0:1], 1.0 / nshards, None, op0=ALU.mult)
    ex2 = sb1.tile([C, 1], F32, tag=f"ex2{lname}")
    nc.vector.tensor_scalar(ex2, ccr[:, 1:2], 1.0 / nshards, None, op0=ALU.mult)
    var = sb1.tile([C, 1], F32, tag=f"var{lname}")
    nc.vector.tensor_tensor(var, mean, mean, op=ALU.mult)
    nc.vector.tensor_tensor(var, ex2, var, op=ALU.subtract)
    sd = sb1.tile([C, 1], F32, tag=f"sd{lname}")
    nc.scalar.activation(sd, var, AF.Sqrt, bias=eps_t, scale=1.0)
    inv = sb1.tile([C, 1], F32, tag=f"inv{lname}")
    nc.vector.reciprocal(out=inv, in_=sd)
    s = sb1.tile([C, 1], F32, tag=f"s{lname}")
    nc.vector.tensor_tensor(s, bnw, inv, op=ALU.mult)
    d = sb1.tile([C, 1], F32, tag=f"d{lname}")
    nc.vector.tensor_tensor(d, mean, s, op=ALU.mult)
    nc.vector.tensor_tensor(d, bnb, d, op=ALU.subtract)
    return s, d


def build():
    nc = bacc.Bacc("TRN2", target_bir_lowering=False, debug=False,
                   num_devices=1 if SIM1 else NCORES)

    # --- I/O ---
    xhi_d = nc.dram_tensor("xhi", [T, BL, C, HW], FP16, kind="ExternalInput")
    xlo_d = None
    if TERMS_L1 >= 3:
        xlo_d = nc.dram_tensor("xlo", [T, BL, C, HW], FP16,
                               kind="ExternalInput")
    nterms = {1: TERMS_L1, 2: TERMS_L2, 3: TERMS_L3}
    w_d = {}
    for l in (1, 2, 3):
        w_d[(l, "hi")] = nc.dram_tensor(f"w{l}hi", [C, 9, C], FP16,
                                        kind="ExternalInput")
        if nterms[l] >= 2:
            w_d[(l, "lo")] = nc.dram_tensor(f"w{l}lo", [C, 9, C], FP16,
                                            kind="ExternalInput")
    bn_d = {}
    for l in (1, 2, 3):
        bn_d[(l, "w")] = nc.dram_tensor(f"bnw{l}", [C, 1], F32,
                                        kind="ExternalInput")
        bn_d[(l, "b")] = nc.dram_tensor(f"bnb{l}", [C, 1], F32,
                                        kind="ExternalInput")
    fcw_d = nc.dram_tensor("fcw", [C, 10], F32, kind="ExternalInput")
    fcb_d = nc.dram_tensor("fcb", [1, 10], F32, kind="ExternalInput")
    out_d = nc.dram_tensor("out", [1, BL * 10], F32, kind="ExternalOutput")
    dbg = {}
    if DEBUG:
        dbg["y1"] = nc.dram_tensor("dbg_y1", [C, HW], F32, kind="ExternalOutput")
        dbg["mv1"] = nc.dram_tensor("dbg_mv1", [C, 2], F32, kind="ExternalOutput")
        dbg["ccr1"] = nc.dram_tensor("dbg_ccr1", [C, 2], F32, kind="ExternalOutput")
        dbg["s1"] = nc.dram_tensor("dbg_s1", [C, 1], F32, kind="ExternalOutput")
        dbg["d1"] = nc.dram_tensor("dbg_d1", [C, 1], F32, kind="ExternalOutput")
        dbg["spk1"] = nc.dram_tensor("dbg_spk1", [C, HW], FP16, kind="ExternalOutput")
        dbg["pool"] = nc.dram_tensor("dbg_pool", [C, T * BL], F32, kind="ExternalOutput")
        dbg["feat"] = nc.dram_tensor("dbg_feat", [C, BL], F32, kind="ExternalOutput")
        dbg["red"] = nc.dram_tensor("dbg_red", [1, BL * 10], F32, kind="ExternalOutput")
        dbg["y2"] = nc.dram_tensor("dbg_y2", [C, HW], F32, kind="ExternalOutput")
        dbg["s2"] = nc.dram_tensor("dbg_s2", [C, 1], F32, kind="ExternalOutput")
        dbg["d2"] = nc.dram_tensor("dbg_d2", [C, 1], F32, kind="ExternalOutput")
        dbg["spk2"] = nc.dram_tensor("dbg_spk2", [C, HW], FP16, kind="ExternalOutput")
        dbg["y3"] = nc.dram_tensor("dbg_y3", [C, HW], F32, kind="ExternalOutput")
        dbg["s3"] = nc.dram_tensor("dbg_s3", [C, 1], F32, kind="ExternalOutput")
        dbg["spk3"] = nc.dram_tensor("dbg_spk3", [C, HW], FP16, kind="ExternalOutput")

    # --- internal DRAM ---
    ydram = nc.dram_tensor("ydram", [C, T, BL, HW], F32)
    cc_bufs = {}
    for l in (1, 2, 3):
        cc_bufs[l] = (
            nc.dram_tensor(f"cc_in{l}", [C, 2], F32),
            nc.dram_tensor(f"cc_out{l}", [C, 2], F32, addr_space="Shared"),
        )

    with ExitStack() as ctx:
        tc = ctx.enter_context(tile.TileContext(nc))
        sb1 = ctx.enter_context(tc.tile_pool(name="sb1", bufs=1))
        sb2 = ctx.enter_context(tc.tile_pool(name="sb2", bufs=2))
        xpool = ctx.enter_context(tc.tile_pool(name="xpool", bufs=2))
        ysb_pool = ctx.enter_context(tc.tile_pool(name="ysb", bufs=3))
        ylif_pool = ctx.enter_context(tc.tile_pool(name="ylif", bufs=2))
        gate_pool = ctx.enter_context(tc.tile_pool(name="gate", bufs=4))
        stage_pool = ctx.enter_context(tc.tile_pool(name="stage", bufs=8))
        mem_pool = ctx.enter_context(tc.tile_pool(name="mem", bufs=1))
        psum_pool = ctx.enter_context(
            tc.tile_pool(name="psum", bufs=8, space="PSUM"))

        # --- load constants ---
        w_sb = {}
        for key, dt_ in w_d.items():
            w_sb[key] = sb1.tile([C, 9, C], FP16,
                                 name=f"w{key[0]}{key[1]}",
                                 tag=f"w{key[0]}{key[1]}")
            nc.sync.dma_start(out=w_sb[key], in_=dt_[:, :, :])
        bn_sb = {}
        for key, dt_ in bn_d.items():
            bn_sb[key] = sb1.tile([C, 1], F32, name=f"bn{key[1]}{key[0]}",
                                  tag=f"bn{key[1]}{key[0]}")
            nc.sync.dma_start(out=bn_sb[key], in_=dt_[:, :])
        fcw_sb = sb1.tile([C, 10], F32)
        nc.sync.dma_start(out=fcw_sb, in_=fcw_d[:, :])
        fcb_sb = sb1.tile([1, 10], F32)
        nc.sync.dma_start(out=fcb_sb, in_=fcb_d[:, :])
        eps_t = sb1.tile([C, 1], F32)
        nc.vector.memset(eps_t, BN_EPS)

        # persistent spike buffers (SBUF-resident): L1->L2 and L2->L3
        spkA = sb1.tile([C, NIMG, HW], FP16)
        spkA3 = spkA.rearrange("c n (h w) -> c n h w", h=H)
        spkB = sb1.tile([C, NIMG, HW], FP16)
        spkB3 = spkB.rearrange("c n (h w) -> c n h w", h=H)
        negth = sb1.tile([C, 1], F32)
        nc.vector.memset(negth, -THRESH)

        # L3 pooled spike counts per (t, b)
        poolbuf = sb1.tile([C, T, BL], F32)

        stats = {}
        for l in (1, 2, 3):
            stats[l] = sb2.tile([C, NIMG * NHALF, 6], F32, name=f"stats{l}",
                                tag="statsbuf")

        # =============== layer 1 conv ===============
        for t in range(T):
            for b in range(BL):
                xhi = xpool.tile([C, HW], FP16, tag="xhi")
                nc.sync.dma_start(out=xhi, in_=xhi_d[t, b, :, :])
                xhi3 = xhi.rearrange("c (h w) -> c h w", h=H)
                terms = [(w_sb[(1, "hi")], xhi3)]
                if TERMS_L1 >= 2:
                    terms.append((w_sb[(1, "lo")], xhi3))
                if TERMS_L1 >= 3:
                    xlo = xpool.tile([C, HW], FP16, tag="xlo")
                    nc.sync.dma_start(out=xlo, in_=xlo_d[t, b, :, :])
                    xlo3 = xlo.rearrange("c (h w) -> c h w", h=H)
                    terms.append((w_sb[(1, "hi")], xlo3))
                img = t * BL + b
                _conv_image(nc, psum_pool, ysb_pool, stats[1], ydram,
                            t, b, terms, img * NHALF)

        s1, d1 = _layer_stats(nc, sb1, stats[1], bn_sb[(1, "w")],
                              bn_sb[(1, "b")], eps_t, *cc_bufs[1], "l1",
                              dbg_mv=dbg.get("mv1"), dbg_ccr=dbg.get("ccr1"))
        if DEBUG:
            nc.sync.dma_start(out=dbg["y1"][:, :], in_=ydram[:, 0, 0, :])
            nc.sync.dma_start(out=dbg["s1"][:, :], in_=s1)
            nc.sync.dma_start(out=dbg["d1"][:, :], in_=d1)

        # =============== LIF helper ===============
        def lif_layer(s, d, spike_sink, gate_a=-DECAY, gate_b=DECAY,
                      add_engs=None, affine_pool=False):
            """spike_sink(t, b, mem_slice) -> AP holding the fp16 spike
            (read back for the gate at t+1). gate = prev*gate_a + gate_b.
            add_engs: per-b engine for the mem+=xn add."""
            if add_engs is None:
                add_engs = [nc.gpsimd] * BL
            mem = mem_pool.tile([C, BL * HW], F32, tag="mem")
            mem4 = mem.rearrange("c (b p) -> c b p", b=BL)
            prev_spk = [None] * BL
            for t in range(T):
                for b in range(BL):
                    yt = ylif_pool.tile([C, HW], F32, tag="ylif")
                    nc.sync.dma_start(out=yt, in_=ydram[:, t, b, :])
                    if t == 0:
                        if affine_pool:
                            nc.gpsimd.tensor_scalar(mem4[:, b], yt, s, d,
                                                    op0=ALU.mult, op1=ALU.add)
                        else:
                            nc.scalar.activation(mem4[:, b], yt, AF.Identity,
                                                 bias=d, scale=s)
                    else:
                        if affine_pool:
                            nc.gpsimd.tensor_scalar(yt, yt, s, d,
                                                    op0=ALU.mult, op1=ALU.add)
                        else:
                            nc.scalar.activation(yt, yt, AF.Identity,
                                                 bias=d, scale=s)
                        gate = gate_pool.tile([C, HW], FP16, tag="gate")
                        nc.gpsimd.tensor_scalar(gate, prev_spk[b],
                                                gate_a, gate_b,
                                                op0=ALU.mult, op1=ALU.add)
                        nc.vector.tensor_tensor(mem4[:, b], mem4[:, b], gate,
                                                op=ALU.mult)
                        add_engs[b].tensor_tensor(mem4[:, b], mem4[:, b], yt,
                                                  op=ALU.add)
                    prev_spk[b] = spike_sink(t, b, mem4[:, b])

        # =============== layer 1 LIF -> spkA ===============
        def sink1(t, b, mem_slice):
            dest = spkA[:, t * BL + b, :]
            nc.vector.tensor_scalar(dest, mem_slice, THRESH, None,
                                    op0=ALU.is_gt)
            return dest

        lif_layer(s1, d1, sink1)
        if DEBUG:
            nc.sync.dma_start(out=dbg["spk1"][:, :], in_=spkA[:, 0, :])

        # =============== layer 2 conv (reads spkA) ===============
        for t in range(T):
            for b in range(BL):
                img = t * BL + b
                x3 = spkA3[:, img]
                terms = [(w_sb[(2, "hi")], x3)]
                if TERMS_L2 >= 2:
                    terms.append((w_sb[(2, "lo")], x3))
                _conv_image(nc, psum_pool, ysb_pool, stats[2], ydram,
                            t, b, terms, img * NHALF)

        s2, d2 = _layer_stats(nc, sb1, stats[2], bn_sb[(2, "w")],
                              bn_sb[(2, "b")], eps_t, *cc_bufs[2], "l2")
        if DEBUG:
            nc.sync.dma_start(out=dbg["y2"][:, :], in_=ydram[:, 0, 0, :])
            nc.sync.dma_start(out=dbg["s2"][:, :], in_=s2)
            nc.sync.dma_start(out=dbg["d2"][:, :], in_=d2)

        # =============== layer 2 LIF -> spkB ===============
        def sink2(t, b, mem_slice):
            dest = spkB[:, t * BL + b, :]
            nc.vector.tensor_scalar(dest, mem_slice, THRESH, None,
                                    op0=ALU.is_gt)
            return dest

        lif_layer(s2, d2, sink2)
        if DEBUG:
            nc.sync.dma_start(out=dbg["spk2"][:, :], in_=spkB[:, 0, :])

        # =============== layer 3 conv (reads spkB) ===============
        for t in range(T):
            for b in range(BL):
                img = t * BL + b
                x3 = spkB3[:, img]
                terms = [(w_sb[(3, "hi")], x3)]
                if TERMS_L3 >= 2:
                    terms.append((w_sb[(3, "lo")], x3))
                _conv_image(nc, psum_pool, ysb_pool, stats[3], ydram,
                            t, b, terms, img * NHALF)

        s3, d3 = _layer_stats(nc, sb1, stats[3], bn_sb[(3, "w")],
                              bn_sb[(3, "b")], eps_t, *cc_bufs[3], "l3")
        if DEBUG:
            nc.sync.dma_start(out=dbg["y3"][:, :], in_=ydram[:, 0, 0, :])
            nc.sync.dma_start(out=dbg["s3"][:, :], in_=s3)

        # ====== layer 3 LIF in u-space (u = mem/s): no per-t affine ======
        # u_t = g*u_{t-1} + y_t + d/s ; spike <=> u > 0.5/s (s > 0)
        sinv = sb1.tile([C, 1], F32)
        nc.vector.reciprocal(out=sinv, in_=s3)
        dp = sb1.tile([C, 1], F32)
        nc.vector.tensor_tensor(dp, d3, sinv, op=ALU.mult)
        negthp = sb1.tile([C, 1], F32)
        nc.vector.tensor_scalar(negthp, sinv, -THRESH, None, op0=ALU.mult)

        umem = mem_pool.tile([C, BL * HW], F32, tag="mem")
        u4 = umem.rearrange("c (b p) -> c b p", b=BL)
        prev_sgn = [None] * BL
        for t in range(T):
            for b in range(BL):
                yt = ylif_pool.tile([C, HW], F32, tag="ylif")
                nc.sync.dma_start(out=yt, in_=ydram[:, t, b, :])
                if t == 0:
                    nc.scalar.activation(u4[:, b], yt, AF.Identity,
                                         bias=dp, scale=1.0)
                else:
                    gate = gate_pool.tile([C, HW], FP16, tag="gate")
                    nc.gpsimd.tensor_scalar(gate, prev_sgn[b],
                                            -DECAY / 2.0, DECAY / 2.0,
                                            op0=ALU.mult, op1=ALU.add)
                    nc.vector.tensor_tensor(u4[:, b], u4[:, b], gate,
                                            op=ALU.mult)
                    # u = (y + d') + u_gated  (one fused DVE op)
                    nc.vector.scalar_tensor_tensor(
                        u4[:, b], yt, dp, u4[:, b],
                        op0=ALU.add, op1=ALU.add)
                st = stage_pool.tile([C, HW], FP16, tag="stage")
                nc.scalar.activation(st, u4[:, b], AF.Sign, bias=negthp,
                                     scale=1.0,
                                     accum_out=poolbuf[:, t, b:b + 1])
                if DEBUG and t == 0 and b == 0:
                    nc.sync.dma_start(out=dbg["spk3"][:, :], in_=st)
                prev_sgn[b] = st
        if DEBUG:
            nc.sync.dma_start(out=dbg["pool"][:, :],
                              in_=poolbuf.rearrange("c t b -> c (t b)"))

        # =============== head: pooling + FC ===============
        feat = sb1.tile([C, BL], F32)
        for b in range(BL):
            nc.vector.tensor_reduce(feat[:, b:b + 1], poolbuf[:, :, b],
                                    axis=mybir.AxisListType.X, op=ALU.add)
        nc.vector.tensor_scalar(feat, feat, 0.5, T * HW / 2.0,
                                op0=ALU.mult, op1=ALU.add)
        # prod[c, b, k] = feat[c, b] * fcw[c, k]
        prod = sb1.tile([C, BL, 10], F32)
        nc.vector.tensor_tensor(
            prod, feat.unsqueeze(2).broadcast_to([C, BL, 10]),
            fcw_sb.unsqueeze(1).broadcast_to([C, BL, 10]), op=ALU.mult)
        if DEBUG:
            nc.sync.dma_start(out=dbg["feat"][:, :], in_=feat)
        red = sb1.tile([C, BL, 10], F32)
        nc.gpsimd.partition_all_reduce(red, prod, channels=C,
                                       reduce_op=bass_isa.ReduceOp.add)
        if DEBUG:
            nc.sync.dma_start(out=dbg["red"][:, :],
                              in_=red[0:1].rearrange("c b k -> c (b k)"))
        ofin = sb1.tile([1, BL, 10], F32)
        nc.vector.tensor_scalar(ofin, red[0:1], 1.0 / POOL_N, None,
                                op0=ALU.mult)
        nc.vector.tensor_tensor(
            ofin, ofin, fcb_sb.unsqueeze(1).broadcast_to([1, BL, 10]),
            op=ALU.add)
        nc.sync.dma_start(out=out_d[:, :],
                          in_=ofin.rearrange("c b k -> c (b k)"))

    nc.compile()
    return nc


_NC_CACHE = {}


def _get_nc():
    if "nc" not in _NC_CACHE:
        _NC_CACHE["nc"] = build()
    return _NC_CACHE["nc"]


def _hi_lo(a):
    hi = a.astype(np.float16)
    lo = (a - hi.astype(np.float32)).astype(np.float16)
    return hi, lo


def make_in_maps(inp, conv_ws, bns, fc_w, fc_b):
    """Build the 8 per-core input maps from full (numpy) model inputs."""
    common = {}
    nterms = {1: TERMS_L1, 2: TERMS_L2, 3: TERMS_L3}
    for li, w in enumerate(conv_ws, start=1):
        wt = np.ascontiguousarray(
            w.transpose(1, 2, 3, 0).reshape(C, 9, C))  # [I, k, O]
        hi, lo = _hi_lo(wt)
        common[f"w{li}hi"] = hi
        if nterms[li] >= 2:
            common[f"w{li}lo"] = lo
        common[f"bnw{li}"] = np.ascontiguousarray(
            bns[li - 1][0].reshape(C, 1))
        common[f"bnb{li}"] = np.ascontiguousarray(
            bns[li - 1][1].reshape(C, 1))
    common["fcw"] = np.ascontiguousarray(fc_w.T)          # [C, 10]
    common["fcb"] = np.ascontiguousarray(fc_b.reshape(1, 10))

    in_maps = []
    for cid in range(NCORES):
        xc = np.ascontiguousarray(
            inp[:, cid * BL:(cid + 1) * BL].reshape(T, BL, C, HW))
        xhi, xlo = _hi_lo(xc)
        m = dict(common)
        m["xhi"] = xhi
        if TERMS_L1 >= 3:
            m["xlo"] = xlo
        in_maps.append(m)
    return in_maps


def kernel(inp, conv_w1, conv_w2, conv_w3, bn_w1, bn_b1, bn_w2, bn_b2,
           bn_w3, bn_b3, fc_w, fc_b):
    inp = np.asarray(inp, dtype=np.float32)
    ws = [np.asarray(w, dtype=np.float32) for w in (conv_w1, conv_w2, conv_w3)]
    bns = [(np.asarray(bn_w1, np.float32), np.asarray(bn_b1, np.float32)),
           (np.asarray(bn_w2, np.float32), np.asarray(bn_b2, np.float32)),
           (np.asarray(bn_w3, np.float32), np.asarray(bn_b3, np.float32))]
    fc_w = np.asarray(fc_w, np.float32)
    fc_b = np.asarray(fc_b, np.float32)

    nc = _get_nc()
    in_maps = make_in_maps(inp, ws, bns, fc_w, fc_b)
    res = run_bass_kernel_spmd(nc, in_maps, core_ids=list(range(NCORES)))
    out = np.concatenate(
        [r["out"].reshape(BL, 10) for r in res.results], axis=0)
    return out.astype(np.float32)



# revision 13
# speedup vs baseline: 2.3400x; 2.3400x over previous
"""Trainium2 Bass kernel for nn_Classifier_64587718197982 (spiking CNN).

Network (per reference):
  3x [conv3x3(C=128, pad=1, no bias) -> TDBN (batchnorm over T,B,H,W) -> LIF]
  -> mean over (H,W) -> mean over T -> FC(128->10)

Sharding: data-parallel over batch B=32 across 8 NeuronCores (4 images/core).
TDBN stats become a tiny [128,2] AllReduce per layer.

v2 design (fused pipeline, SBUF-resident):
  - y for all 32 images of the current layer lives in ONE SBUF f32 buffer
    [C, 32, HW] (128KB/partition); layer l+1's conv output overwrites slot
    (t,b) only after layer l's LIF consumed it (WAR tracked by Tile).
  - Phases: A = conv1 (all imgs); B = LIF1+conv2 interleaved per image;
    C = LIF2+conv3; D = LIF3+pool. TensorE never waits on LIF except at the
    3 BN-stats barriers.
  - conv: per image one [C,1024] PSUM tile (2 banks, halves bank-aligned);
    3x3 conv = 9 range-restricted shifted matmuls per half in fp16 (weights
    hi-rounded; spikes exact in fp16).
  - BN stats: Sum(y) rides the ScalarE PSUM->SBUF drain via accum_out;
    Sum(y^2) is one ScalarE Square (output to a PSUM scratch bank,
    accum_out -> slot). AllReduce of [Sum, SumSq]; then
    s' = 2*bn_w*rsqrt(var+eps), d' = 2*(bn_b - mean*s)  (r = mem/0.5 space,
    so the spike threshold is uniformly 1.0; x2 scaling is exact).
  - LIF (r-space, per image): z = s'*y + d' (ScalarE, per-partition
    scale/bias); gate g = 0.25*(1-S_prev) (GpSimd from fp16 spike);
    u *= g; u += z (VectorE); S = (u > 1) -> fp16 (VectorE). Spikes live in
    a 2-deep (t, t-1) ring per b-slot.
  - Phase D needs no spikes for a conv: gate+pool fuse into one VectorE op
    g = (u <= 1)*0.25 with accum_out counting non-spikes.
  - head: feat = 1 - negcount*4/8192; logits = fcw.T @ feat (one tiny
    matmul) + fcb via ScalarE bias drain; output [10, BL], transposed on
    host.
"""
import numpy as np
import ml_dtypes
from contextlib import ExitStack

import concourse.bass as bass
import concourse.mybir as mybir
import concourse.tile as tile
from concourse import bass_isa
from concourse import bacc
from concourse.bass_utils import run_bass_kernel_spmd

F32 = mybir.dt.float32
FP16 = mybir.dt.float16
AF = mybir.ActivationFunctionType
ALU = mybir.AluOpType

T, B, C, H, W = 8, 32, 128, 32, 32
NCORES = 8
BL = B // NCORES          # images per core per timestep
HW = H * W                # 1024
NIMG = T * BL             # 32 images per core
NHALF = 2                 # psum banks per image (16 rows x 32 cols = 512)
RH = H // NHALF           # rows per half
DECAY = 0.25
THRESH = 0.5
BN_EPS = 1e-5
POOL_N = float(T * HW)    # pooling divisor 8192

# fp16 term counts for L1 conv (1 = single fp16 term; 3 = ~fp32 via hi/lo)
TERMS_L1 = 1
STATS_IMGS = NIMG         # images contributing to BN stats (<32 = sampled)
D_FP16 = True             # phase-D membrane dtype fp16 (L3 only; no conv after)
DEBUG = False

# conv shift order: (1,1) first so the start=True matmul covers the full half
SHIFTS = [(1, 1)] + [(dy, dx) for dy in range(3) for dx in range(3)
                     if not (dy == 1 and dx == 1)]


def _conv_image(nc, psum_pool, ybuf, sum_sl, sq_sl, img, terms):
    """One image conv: terms = list of (w_sb[C,9,C], x3d[C,H,W]).
    Drains f32 y into ybuf[:, img] with Sum(y) accum; Sum(y^2) via Square."""
    pt = psum_pool.tile([C, HW], F32, tag="psum")
    p3 = pt.rearrange("c (r w) -> c r w", r=H)
    for h in range(NHALF):
        r_base = h * RH
        n_mm = len(SHIFTS) * len(terms)
        i_mm = 0
        for (dy, dx) in SHIFTS:
            oy, ox = dy - 1, dx - 1
            r0 = max(r_base, -oy)
            r1 = min(r_base + RH, H - oy)
            c0 = max(0, -ox)
            c1 = min(W, W - ox)
            k = dy * 3 + dx
            for (w_sb, x3d) in terms:
                nc.tensor.matmul(
                    p3[:, r0:r1, c0:c1],
                    w_sb[:, k, :],
                    x3d[:, r0 + oy:r1 + oy, c0 + ox:c1 + ox],
                    start=(i_mm == 0), stop=(i_mm == n_mm - 1),
                )
                i_mm += 1
    if img < STATS_IMGS:
        nc.scalar.activation(ybuf[:, img], pt, AF.Identity,
                             accum_out=sum_sl[:, img:img + 1])
        nc.scalar.activation(pt, pt, AF.Square,
                             accum_out=sq_sl[:, img:img + 1])
    else:
        nc.scalar.copy(out=ybuf[:, img], in_=pt)


def _layer_stats(nc, sb1, sum_sl, sq_sl, bnw, bnb, eps_t, cc_in, cc_out,
                 lname, dbg_cc=None, dbg_ccr=None):
    """Reduce per-image sums, AllReduce, compute s' = 2s and d' = 2d."""
    cc = sb1.tile([C, 2], F32, tag=f"cc{lname}")
    nc.vector.tensor_reduce(cc[:, 0:1], sum_sl,
                            axis=mybir.AxisListType.X, op=ALU.add)
    nc.vector.tensor_reduce(cc[:, 1:2], sq_sl,
                            axis=mybir.AxisListType.X, op=ALU.add)
    if dbg_cc is not None:
        nc.sync.dma_start(out=dbg_cc[:, :], in_=cc)
    nc.sync.dma_start(out=cc_in[:, :], in_=cc)
    nc.gpsimd.collective_compute(
        "AllReduce", ALU.add,
        replica_groups=[list(range(NCORES))],
        ins=[cc_in[:, :]], outs=[cc_out[:, :]],
    )
    ccr = sb1.tile([C, 2], F32, tag=f"ccr{lname}")
    nc.sync.dma_start(out=ccr, in_=cc_out[:, :])
    if dbg_ccr is not None:
        nc.sync.dma_start(out=dbg_ccr[:, :], in_=ccr)
    n_tot = float(STATS_IMGS * HW * NCORES)
    mean = sb1.tile([C, 1], F32, tag=f"mean{lname}")
    nc.vector.tensor_scalar(mean, ccr[:, 0:1], 1.0 / n_tot, None,
                            op0=ALU.mult)
    ex2 = sb1.tile([C, 1], F32, tag=f"ex2{lname}")
    nc.vector.tensor_scalar(ex2, ccr[:, 1:2], 1.0 / n_tot, None,
                            op0=ALU.mult)
    var = sb1.tile([C, 1], F32, tag=f"var{lname}")
    nc.vector.tensor_tensor(var, mean, mean, op=ALU.mult)
    nc.vector.tensor_tensor(var, ex2, var, op=ALU.subtract)
    sd = sb1.tile([C, 1], F32, tag=f"sd{lname}")
    nc.scalar.activation(sd, var, AF.Sqrt, bias=eps_t, scale=1.0)
    inv = sb1.tile([C, 1], F32, tag=f"inv{lname}")
    nc.vector.reciprocal(out=inv, in_=sd)
    s1 = sb1.tile([C, 1], F32, tag=f"s1{lname}")
    nc.vector.tensor_tensor(s1, bnw, inv, op=ALU.mult)
    sp = sb1.tile([C, 1], F32, tag=f"sp{lname}")
    nc.vector.tensor_scalar(sp, s1, 2.0, None, op0=ALU.mult)
    ms = sb1.tile([C, 1], F32, tag=f"ms{lname}")
    nc.vector.tensor_tensor(ms, mean, s1, op=ALU.mult)
    dp = sb1.tile([C, 1], F32, tag=f"dp{lname}")
    nc.vector.tensor_tensor(dp, bnb, ms, op=ALU.subtract)
    nc.vector.tensor_scalar(dp, dp, 2.0, None, op0=ALU.mult)
    return sp, dp


def build():
    nc = bacc.Bacc("TRN2", target_bir_lowering=False, debug=False,
                   num_devices=NCORES)

    # --- I/O ---
    xhi_d = nc.dram_tensor("xhi", [T, BL, C, HW], FP16, kind="ExternalInput")
    xlo_d = None
    if TERMS_L1 >= 3:
        xlo_d = nc.dram_tensor("xlo", [T, BL, C, HW], FP16,
                               kind="ExternalInput")
    w_d = {}
    for l in (1, 2, 3):
        w_d[(l, "hi")] = nc.dram_tensor(f"w{l}hi", [C, 9, C], FP16,
                                        kind="ExternalInput")
    if TERMS_L1 >= 2:
        w_d[(1, "lo")] = nc.dram_tensor("w1lo", [C, 9, C], FP16,
                                        kind="ExternalInput")
    bn_d = {}
    for l in (1, 2, 3):
        bn_d[(l, "w")] = nc.dram_tensor(f"bnw{l}", [C, 1], F32,
                                        kind="ExternalInput")
        bn_d[(l, "b")] = nc.dram_tensor(f"bnb{l}", [C, 1], F32,
                                        kind="ExternalInput")
    fcw_d = nc.dram_tensor("fcw", [C, 10], F32, kind="ExternalInput")
    fcb_d = nc.dram_tensor("fcb", [10, 1], F32, kind="ExternalInput")
    out_d = nc.dram_tensor("out", [10, BL], F32, kind="ExternalOutput")
    dbg = {}
    if DEBUG:
        for nm, shp in (("y1", [C, HW]), ("cc1", [C, 2]), ("ccr1", [C, 2]),
                        ("sp1", [C, 1]), ("dp1", [C, 1]), ("spk1", [C, HW]),
                        ("y2", [C, HW]), ("sp2", [C, 1]), ("dp2", [C, 1]),
                        ("spk2", [C, HW]), ("y3", [C, HW]),
                        ("pool", [C, T * BL]), ("feat", [C, BL])):
            dt_ = FP16 if nm.startswith("spk") else F32
            dbg[nm] = nc.dram_tensor(f"dbg_{nm}", shp, dt_,
                                     kind="ExternalOutput")

    # --- internal DRAM for collectives ---
    cc_bufs = {}
    for l in (1, 2, 3):
        cc_bufs[l] = (
            nc.dram_tensor(f"cc_in{l}", [C, 2], F32),
            nc.dram_tensor(f"cc_out{l}", [C, 2], F32, addr_space="Shared"),
        )

    with ExitStack() as ctx:
        tc = ctx.enter_context(tile.TileContext(nc))
        sb1 = ctx.enter_context(tc.tile_pool(name="sb1", bufs=1))
        xpool = ctx.enter_context(tc.tile_pool(name="xpool", bufs=2))
        zpool = ctx.enter_context(tc.tile_pool(name="zpool", bufs=2))
        gpool = ctx.enter_context(tc.tile_pool(name="gpool", bufs=2))
        psum_pool = ctx.enter_context(
            tc.tile_pool(name="psum", bufs=3, space="PSUM"))
        fcp_pool = ctx.enter_context(
            tc.tile_pool(name="fcp", bufs=1, space="PSUM"))

        # --- load constants ---
        w_sb = {}
        for key, dt_ in w_d.items():
            w_sb[key] = sb1.tile([C, 9, C], FP16, name=f"w{key[0]}{key[1]}",
                                 tag=f"w{key[0]}{key[1]}")
            nc.sync.dma_start(out=w_sb[key], in_=dt_[:, :, :])
        bn_sb = {}
        for key, dt_ in bn_d.items():
            bn_sb[key] = sb1.tile([C, 1], F32, name=f"bn{key[1]}{key[0]}",
                                  tag=f"bn{key[1]}{key[0]}")
            nc.sync.dma_start(out=bn_sb[key], in_=dt_[:, :])
        fcw_sb = sb1.tile([C, 10], F32, tag="fcw")
        nc.sync.dma_start(out=fcw_sb, in_=fcw_d[:, :])
        fcb_sb = sb1.tile([10, 1], F32, tag="fcb")
        nc.sync.dma_start(out=fcb_sb, in_=fcb_d[:, :])
        eps_t = sb1.tile([C, 1], F32, tag="eps")
        nc.vector.memset(eps_t, BN_EPS)

        # --- persistent state ---
        ybuf = sb1.tile([C, NIMG, HW], F32, tag="ybuf")      # 128KB/part
        umem = sb1.tile([C, BL, HW], F32, tag="umem")        # 16KB/part
        if D_FP16:
            udm = sb1.tile([C, BL, HW], FP16, tag="udm")     # 8KB/part
        else:
            udm = umem
        ring = sb1.tile([C, 2, BL, HW], FP16, tag="ring")    # 16KB/part
        ring4 = ring.rearrange("c s b (h w) -> c s b h w", h=H)
        sum_sl = {}
        sq_sl = {}
        for l in (1, 2, 3):
            sum_sl[l] = sb1.tile([C, STATS_IMGS], F32, name=f"sum{l}",
                                 tag=f"sum{l}")
            sq_sl[l] = sb1.tile([C, STATS_IMGS], F32, name=f"sq{l}",
                                tag=f"sq{l}")
        poolneg = sb1.tile([C, T, BL], F32, tag="poolneg")

        # =============== phase A: conv L1 ===============
        for t in range(T):
            for b in range(BL):
                xhi = xpool.tile([C, HW], FP16, tag="xhi")
                nc.sync.dma_start(out=xhi, in_=xhi_d[t, b, :, :])
                xhi3 = xhi.rearrange("c (h w) -> c h w", h=H)
                terms = [(w_sb[(1, "hi")], xhi3)]
                if TERMS_L1 >= 2:
                    terms.append((w_sb[(1, "lo")], xhi3))
                if TERMS_L1 >= 3:
                    xlo = xpool.tile([C, HW], FP16, tag="xlo")
                    nc.sync.dma_start(out=xlo, in_=xlo_d[t, b, :, :])
                    terms.append((w_sb[(1, "hi")],
                                  xlo.rearrange("c (h w) -> c h w", h=H)))
                _conv_image(nc, psum_pool, ybuf,
                            sum_sl[1], sq_sl[1], t * BL + b, terms)

        sp1, dp1 = _layer_stats(nc, sb1, sum_sl[1], sq_sl[1],
                                bn_sb[(1, "w")], bn_sb[(1, "b")], eps_t,
                                *cc_bufs[1], "l1",
                                dbg_cc=dbg.get("cc1"),
                                dbg_ccr=dbg.get("ccr1"))
        if DEBUG:
            nc.sync.dma_start(out=dbg["y1"][:, :], in_=ybuf[:, 0])
            nc.sync.dma_start(out=dbg["sp1"][:, :], in_=sp1)
            nc.sync.dma_start(out=dbg["dp1"][:, :], in_=dp1)

        # =============== fused LIF + next-layer conv ===============
        def lif_conv_phase(sp, dp, wkey, dbg_spk=None):
            """LIF for the layer whose y is in ybuf, spikes -> ring, and
            immediately conv the spike image into ybuf[:, img]."""
            for t in range(T):
                for b in range(BL):
                    img = t * BL + b
                    if t == 0:
                        nc.scalar.activation(umem[:, b], ybuf[:, img],
                                             AF.Identity, bias=dp, scale=sp)
                    else:
                        z = zpool.tile([C, HW], F32, tag="z")
                        nc.scalar.activation(z, ybuf[:, img], AF.Identity,
                                             bias=dp, scale=sp)
                        g = gpool.tile([C, HW], FP16, tag="g")
                        nc.gpsimd.tensor_scalar(g, ring[:, (t - 1) % 2, b],
                                                -DECAY, DECAY,
                                                op0=ALU.mult, op1=ALU.add)
                        nc.vector.tensor_tensor(umem[:, b], umem[:, b], g,
                                                op=ALU.mult)
                        nc.vector.tensor_tensor(umem[:, b], z, umem[:, b],
                                                op=ALU.add)
                    S = ring[:, t % 2, b]
                    nc.vector.tensor_scalar(S, umem[:, b], 1.0, None,
                                            op0=ALU.is_gt)
                    if dbg_spk is not None and t == 0 and b == 0:
                        nc.sync.dma_start(out=dbg_spk[:, :], in_=S)
                    _conv_image(nc, psum_pool, ybuf,
                                sum_sl[wkey], sq_sl[wkey], img,
                                [(w_sb[(wkey, "hi")], ring4[:, t % 2, b])])

        # phase B: LIF1 + conv2
        lif_conv_phase(sp1, dp1, 2,
                       dbg_spk=dbg.get("spk1"))
        sp2, dp2 = _layer_stats(nc, sb1, sum_sl[2], sq_sl[2],
                                bn_sb[(2, "w")], bn_sb[(2, "b")], eps_t,
                                *cc_bufs[2], "l2")
        if DEBUG:
            nc.sync.dma_start(out=dbg["y2"][:, :], in_=ybuf[:, 0])
            nc.sync.dma_start(out=dbg["sp2"][:, :], in_=sp2)
            nc.sync.dma_start(out=dbg["dp2"][:, :], in_=dp2)
        # phase C: LIF2 + conv3
        lif_conv_phase(sp2, dp2, 3,
                       dbg_spk=dbg.get("spk2"))
        sp3, dp3 = _layer_stats(nc, sb1, sum_sl[3], sq_sl[3],
                                bn_sb[(3, "w")], bn_sb[(3, "b")], eps_t,
                                *cc_bufs[3], "l3")
        if DEBUG:
            nc.sync.dma_start(out=dbg["y3"][:, :], in_=ybuf[:, 0])

        # =============== phase D: LIF3 + pooled non-spike counts ==========
        ZD = FP16 if D_FP16 else F32
        for t in range(T):
            for b in range(BL):
                img = t * BL + b
                if t == 0:
                    nc.scalar.activation(udm[:, b], ybuf[:, img],
                                         AF.Identity, bias=dp3, scale=sp3)
                else:
                    z = zpool.tile([C, HW], ZD, tag="zd")
                    nc.scalar.activation(z, ybuf[:, img], AF.Identity,
                                         bias=dp3, scale=sp3)
                    nc.vector.tensor_tensor(udm[:, b], udm[:, b],
                                            ring[:, (t - 1) % 2, b],
                                            op=ALU.mult)
                    nc.vector.tensor_tensor(udm[:, b], z, udm[:, b],
                                            op=ALU.add)
                # sg = sign(1 - u): +1 no-spike, -1 spike; accum = #non - #spk
                sg = ring[:, t % 2, b]
                nc.scalar.activation(sg, udm[:, b], AF.Sign,
                                     bias=1.0, scale=-1.0,
                                     accum_out=poolneg[:, t, b:b + 1])
                # in-place: gate = 0.125*sg + 0.125 in {0.25, 0}
                nc.gpsimd.tensor_scalar(sg, sg, 0.125, 0.125,
                                        op0=ALU.mult, op1=ALU.add)

        if DEBUG:
            nc.sync.dma_start(
                out=dbg["pool"][:, :],
                in_=poolneg.rearrange("c t b -> c (t b)"))
        # =============== head: pooling + FC ===============
        negfeat = sb1.tile([C, BL], F32, tag="negfeat")
        for b in range(BL):
            nc.vector.tensor_reduce(negfeat[:, b:b + 1], poolneg[:, :, b],
                                    axis=mybir.AxisListType.X, op=ALU.add)
        # accum counted sign(1-u): q = #non - #spk; spike_frac = 0.5 - q/16384
        feat = sb1.tile([C, BL], F32, tag="feat")
        nc.vector.tensor_scalar(feat, negfeat, -1.0 / (2 * POOL_N), 0.5,
                                op0=ALU.mult, op1=ALU.add)
        if DEBUG:
            nc.sync.dma_start(out=dbg["feat"][:, :], in_=feat)
        pfc = fcp_pool.tile([10, BL], F32, tag="pfc")
        nc.tensor.matmul(pfc, fcw_sb, feat, start=True, stop=True)
        ofin = sb1.tile([10, BL], F32, tag="ofin")
        nc.scalar.activation(ofin, pfc, AF.Identity, bias=fcb_sb, scale=1.0)
        nc.sync.dma_start(out=out_d[:, :], in_=ofin)

    nc.compile()
    return nc


_NC_CACHE = {}


def _get_nc():
    if "nc" not in _NC_CACHE:
        _NC_CACHE["nc"] = build()
    return _NC_CACHE["nc"]


def _hi_lo(a):
    hi = a.astype(np.float16)
    lo = (a - hi.astype(np.float32)).astype(np.float16)
    return hi, lo


def make_in_maps(inp, conv_ws, bns, fc_w, fc_b):
    """Build the 8 per-core input maps from full (numpy) model inputs."""
    common = {}
    for li, w in enumerate(conv_ws, start=1):
        wt = np.ascontiguousarray(
            w.transpose(1, 2, 3, 0).reshape(C, 9, C))  # [I, k, O]
        hi, lo = _hi_lo(wt)
        common[f"w{li}hi"] = hi
        if li == 1 and TERMS_L1 >= 2:
            common["w1lo"] = lo
        common[f"bnw{li}"] = np.ascontiguousarray(
            bns[li - 1][0].reshape(C, 1))
        common[f"bnb{li}"] = np.ascontiguousarray(
            bns[li - 1][1].reshape(C, 1))
    common["fcw"] = np.ascontiguousarray(fc_w.T)          # [C, 10]
    common["fcb"] = np.ascontiguousarray(fc_b.reshape(10, 1))

    in_maps = []
    for cid in range(NCORES):
        xc = np.ascontiguousarray(
            inp[:, cid * BL:(cid + 1) * BL].reshape(T, BL, C, HW))
        xhi, xlo = _hi_lo(xc)
        m = dict(common)
        m["xhi"] = xhi
        if TERMS_L1 >= 3:
            m["xlo"] = xlo
        in_maps.append(m)
    return in_maps


def kernel(inp, conv_w1, conv_w2, conv_w3, bn_w1, bn_b1, bn_w2, bn_b2,
           bn_w3, bn_b3, fc_w, fc_b):
    inp = np.asarray(inp, dtype=np.float32)
    ws = [np.asarray(w, dtype=np.float32) for w in (conv_w1, conv_w2, conv_w3)]
    bns = [(np.asarray(bn_w1, np.float32), np.asarray(bn_b1, np.float32)),
           (np.asarray(bn_w2, np.float32), np.asarray(bn_b2, np.float32)),
           (np.asarray(bn_w3, np.float32), np.asarray(bn_b3, np.float32))]
    fc_w = np.asarray(fc_w, np.float32)
    fc_b = np.asarray(fc_b, np.float32)

    nc = _get_nc()
    in_maps = make_in_maps(inp, ws, bns, fc_w, fc_b)
    res = run_bass_kernel_spmd(nc, in_maps, core_ids=list(range(NCORES)))
    out = np.concatenate(
        [r["out"].reshape(10, BL).T for r in res.results], axis=0)
    return out.astype(np.float32)
